# revision 19
# baseline (speedup 1.0000x reference)
"""Trainium2 Bass kernel: GroupNorm + single-head self-attention block.

Reference computation (per batch b, x: [C=512, HW=1024] after flattening spatial):
    xn   = groupnorm(x, 8 groups over C, eps=1e-5) * gamma + beta
    qkv  = qkv_w @ xn + qkv_b            # [3C, HW]
    sT   = k^T q * C^-0.5                # computed transposed: [j, i]
    e    = exp(sT)                       # softmax without max-subtraction
    colsum[i] = sum_j e[j, i]
    oun  = v @ e                         # [C, i]
    res  = out_w @ oun + out_b
    out  = x + res / colsum              # residual; softmax denom applied last

Sharding: data-parallel over batch, 32 batches / 8 cores = 4 per core.
All large matmuls run as float32r (fp32 storage, single-pass PE streaming);
measured end-to-end absmax error vs the fp32 reference is ~3e-3 under a
worst-case bf16-rounding model of float32r.
"""

import json
import os

import numpy as np

import concourse.bass as bass
import concourse.mybir as mybir
import concourse.tile as tile
from concourse.bass_utils import run_bass_kernel_spmd


def _spill_multiwaits(raw: bytes) -> bytes:
    """Walrus in this toolchain accepts only one sync-wait command per
    instruction descriptor. Spill extra on_wait entries onto single-wait
    EventSemaphore instructions inserted immediately before, on the same
    engine queue (the exact pattern Tile's own barriers use), which is
    semantically identical: the queue blocks at the same point either way.
    """
    j = json.loads(raw)
    n = 0
    for fn in j.get("functions", []):
        for blk in fn.get("blocks", []):
            out = []
            for inst in blk.get("instructions", []):
                si = inst.get("sync_info") or {}
                waits = si.get("on_wait") or []
                if len(waits) > 1 and inst.get("engine"):
                    for spilled in waits[:-1]:
                        n += 1
                        out.append({
                            "debug": inst.get("debug", 0),
                            "engine": inst["engine"],
                            "ins": [],
                            "name": f"{inst['name']}-sw{n}",
                            "opcode": "EventSemaphore",
                            "outs": [],
                            "sync_info": {"on_update": [], "on_wait": [spilled]},
                        })
                    si["on_wait"] = waits[-1:]
                out.append(inst)
            blk["instructions"] = out
    return json.dumps(j).encode()


_orig_to_json_bytes = bass.Bass.to_json_bytes


def _patched_to_json_bytes(self):
    return _spill_multiwaits(_orig_to_json_bytes(self))


bass.Bass.to_json_bytes = _patched_to_json_bytes

F32 = mybir.dt.float32
MM_DT = mybir.dt.float32r

N_CORES = 8
B_TOTAL = 32
B_PER_CORE = B_TOTAL // N_CORES
C = 512
HW = 1024
GROUPS = 8
EPS = 1e-5
SCALE = float(C) ** -0.5

CT = C // 128   # 4 channel tiles
PT = HW // 128  # 8 pixel tiles
NB = HW // 512  # 2 free-dim blocks of 512


def build_nc():
    nc = bass.Bass()

    x_d = nc.dram_tensor("x", [B_PER_CORE, C, HW], F32, kind="ExternalInput")
    wqkT_d = nc.dram_tensor("wqkT", [C, 2 * C], MM_DT, kind="ExternalInput")
    wvT_d = nc.dram_tensor("wvT", [C, C], MM_DT, kind="ExternalInput")
    woT_d = nc.dram_tensor("woT", [C, C], MM_DT, kind="ExternalInput")
    qkb_d = nc.dram_tensor("qkb", [2 * C], F32, kind="ExternalInput")
    outb_d = nc.dram_tensor("outb", [C], F32, kind="ExternalInput")
    gamma_d = nc.dram_tensor("gamma", [C], F32, kind="ExternalInput")
    beta_d = nc.dram_tensor("beta", [C], F32, kind="ExternalInput")
    sel_d = nc.dram_tensor("sel", [C, GROUPS], F32, kind="ExternalInput")
    selT_d = nc.dram_tensor("selT", [GROUPS, C], F32, kind="ExternalInput")
    out_d = nc.dram_tensor("out", [B_PER_CORE, C, HW], F32, kind="ExternalOutput")
    warmdump_d = nc.dram_tensor("warmdump", [128, 4], F32)

    with tile.TileContext(nc) as tc:
        with (
            tc.tile_pool(name="wpool", bufs=1) as wpool,
            tc.tile_pool(name="xpool", bufs=2) as xpool,
            tc.tile_pool(name="xnpool", bufs=1) as xnpool,
            tc.tile_pool(name="qkpool", bufs=1) as qkpool,
            tc.tile_pool(name="vtpool", bufs=1) as vtpool,
            tc.tile_pool(name="expool", bufs=1) as expool,
            tc.tile_pool(name="ounpool", bufs=1) as ounpool,
            tc.tile_pool(name="rpool", bufs=1) as rpool,
            tc.tile_pool(name="spool", bufs=2) as spool,
            tc.tile_pool(name="ftpool", bufs=2) as ftpool,
            tc.tile_pool(name="mmps", bufs=4, space=bass.MemorySpace.PSUM) as mmps,
            tc.tile_pool(name="colps", bufs=1, space=bass.MemorySpace.PSUM) as colpool,
            tc.tile_pool(name="stps", bufs=1, space=bass.MemorySpace.PSUM) as stps,
        ):
            xts = {}

            def load_x(bb):
                xt = xpool.tile([128, CT, HW], F32, tag="xt")
                xts[bb] = xt
                # per-c-tile chunks so bn_stats can start before the full load
                for t in range(CT):
                    nc.sync.dma_start(
                        out=xt[:, t],
                        in_=x_d[bb, t * 128:(t + 1) * 128, :])
                return xt

            # x(0) first: its consumer chain (stats -> xn -> qkv) is the
            # critical path to the first big matmul
            load_x(0)

            # ---- tiny constants (cheap DMAs / memsets) ----
            eps_sb = wpool.tile([128, 1], F32)
            nc.vector.memset(eps_sb, EPS)
            ones_st = wpool.tile([128, 128], F32)
            nc.vector.memset(ones_st, 1.0)
            ones_sb = wpool.tile([128, 128], MM_DT)
            nc.vector.tensor_copy(ones_sb, ones_st)
            # HAM warmup: keep the PE busy under the startup DMA window so the
            # clock gate reaches 8/8 (2.4GHz) before the first real matmul,
            # instead of ~40 real matmuls running at the cold 1.2GHz rate.
            warm_st = wpool.tile([128, 512], F32)
            nc.vector.memset(warm_st, 0.0)
            warm_rhs = wpool.tile([128, 512], MM_DT)
            nc.vector.tensor_copy(warm_rhs, warm_st)
            warm_ps = stps.tile([128, 512], F32, tag="gps")
            for w in range(70):
                nc.tensor.matmul(warm_ps, lhsT=ones_sb, rhs=warm_rhs,
                                 start=True, stop=True)
            warm_out = wpool.tile([128, 4], F32)
            nc.vector.tensor_copy(warm_out, warm_ps[:, 0:4])
            nc.sync.dma_start(out=warmdump_d[:, :], in_=warm_out)
            sel_st = wpool.tile([128, CT, GROUPS], F32)
            nc.sync.dma_start(out=sel_st, in_=sel_d.rearrange("(t p) g -> p t g", p=128))
            sel_sb = wpool.tile([128, CT, GROUPS], MM_DT)
            nc.vector.tensor_copy(sel_sb, sel_st)
            selT_st = wpool.tile([GROUPS, C], F32)
            nc.sync.dma_start(out=selT_st, in_=selT_d[:, :])
            selT_sb = wpool.tile([GROUPS, C], MM_DT)
            nc.vector.tensor_copy(selT_sb, selT_st)
            qkb_sb = wpool.tile([128, 2 * CT], F32)
            nc.sync.dma_start(out=qkb_sb, in_=qkb_d.rearrange("(m p) -> p m", p=128))
            outb_sb = wpool.tile([128, CT], F32)
            nc.sync.dma_start(out=outb_sb, in_=outb_d.rearrange("(m p) -> p m", p=128))
            gamma_sb = wpool.tile([128, CT], F32)
            nc.sync.dma_start(out=gamma_sb, in_=gamma_d.rearrange("(m p) -> p m", p=128))
            beta_sb = wpool.tile([128, CT], F32)
            nc.sync.dma_start(out=beta_sb, in_=beta_d.rearrange("(m p) -> p m", p=128))

            # ---- weights (split per c-tile for queue parallelism) ----
            wqk_sb = wpool.tile([128, CT, 2 * C], MM_DT)
            wv_sb = wpool.tile([128, CT, C], MM_DT)
            wo_sb = wpool.tile([128, CT, C], MM_DT)
            wqkT_r = wqkT_d.rearrange("(t p) o -> p t o", p=128)
            wvT_r = wvT_d.rearrange("(t p) o -> p t o", p=128)
            woT_r = woT_d.rearrange("(t p) o -> p t o", p=128)
            for t in range(CT):
                nc.sync.dma_start(out=wqk_sb[:, t], in_=wqkT_r[:, t])
            for t in range(CT):
                nc.sync.dma_start(out=wv_sb[:, t], in_=wvT_r[:, t])
                nc.sync.dma_start(out=wo_sb[:, t], in_=woT_r[:, t])

            def norm_stage(bb):
                """GroupNorm stats + normalized activations for batch bb."""
                xt = xts[bb]
                stats3 = spool.tile([128, CT, 4], F32, tag="stats3")
                nc.vector.memset(stats3, 0.0)
                for t in range(CT):
                    st6 = spool.tile([128, 2, 6], F32, tag="st6")
                    for sg in range(2):
                        nc.vector.bn_stats(out=st6[:, sg], in_=xt[:, t, sg * 512:(sg + 1) * 512])
                    nc.vector.bn_aggr(out=stats3[:, t, 0:2], in_=st6)
                    nc.vector.tensor_mul(stats3[:, t, 2:3], stats3[:, t, 0:1], stats3[:, t, 0:1])
                stats3r = spool.tile([128, CT, 4], MM_DT, tag="stats3r")
                nc.vector.tensor_copy(stats3r, stats3)
                gps = stps.tile([GROUPS, 4], F32, tag="gps")
                for t in range(CT):
                    nc.tensor.matmul(gps, lhsT=sel_sb[:, t], rhs=stats3r[:, t],
                                     start=(t == 0), stop=(t == CT - 1))
                # group var = E[var_c] + E[mean_c^2] - E[mean_c]^2 ; then rstd
                gsb = spool.tile([GROUPS, 4], F32, tag="gsb")
                nc.vector.tensor_copy(gsb, gps)
                gs = spool.tile([GROUPS, 4], F32, tag="gs")
                nc.vector.memset(gs, 0.0)
                tmp8 = spool.tile([GROUPS, 1], F32, tag="tmp8")
                nc.vector.tensor_mul(tmp8, gsb[:, 0:1], gsb[:, 0:1])
                nc.vector.tensor_add(gs[:, 1:2], gsb[:, 1:2], gsb[:, 2:3])
                nc.vector.tensor_sub(gs[:, 1:2], gs[:, 1:2], tmp8)
                nc.scalar.activation(gs[:, 1:2], gs[:, 1:2],
                                     mybir.ActivationFunctionType.Sqrt,
                                     bias=eps_sb[:GROUPS])
                nc.vector.reciprocal(gs[:, 1:2], gs[:, 1:2])
                nc.vector.tensor_copy(gs[:, 0:1], gsb[:, 0:1])
                # broadcast group stats back to channel partitions
                gsr = spool.tile([GROUPS, 4], MM_DT, tag="gsr")
                nc.vector.tensor_copy(gsr, gs)
                csps = stps.tile([128, CT, 4], F32, tag="csps")
                for t in range(CT):
                    nc.tensor.matmul(csps[:, t], lhsT=selT_sb[:, t * 128:(t + 1) * 128],
                                     rhs=gsr, start=True, stop=True)
                # per-channel affine: xn = x * s + tt
                stv = spool.tile([128, CT, 2], F32, tag="stv")
                for t in range(CT):
                    tmpc = spool.tile([128, 1], F32, tag="tmpc")
                    nc.vector.tensor_mul(stv[:, t, 0:1], csps[:, t, 1:2], gamma_sb[:, t:t + 1])
                    nc.vector.tensor_mul(tmpc, csps[:, t, 0:1], stv[:, t, 0:1])
                    nc.vector.tensor_sub(stv[:, t, 1:2], beta_sb[:, t:t + 1], tmpc)
                xn = xnpool.tile([128, CT, HW], MM_DT, tag="xn")
                for t in range(CT):
                    nc.vector.tensor_scalar(out=xn[:, t], in0=xt[:, t],
                                            scalar1=stv[:, t, 0:1], scalar2=stv[:, t, 1:2],
                                            op0=mybir.AluOpType.mult,
                                            op1=mybir.AluOpType.add)
                return xn

            def att_part1(bb, xn):
                """qkv projections for batch bb."""
                qk = qkpool.tile([128, 2 * CT, HW], MM_DT, tag="qk")
                for m in range(2 * CT):
                    for n in range(NB):
                        ps = mmps.tile([128, 512], F32, tag="mm")
                        for t in range(CT):
                            nc.tensor.matmul(ps,
                                             lhsT=wqk_sb[:, t, m * 128:(m + 1) * 128],
                                             rhs=xn[:, t, n * 512:(n + 1) * 512],
                                             start=(t == 0), stop=(t == CT - 1))
                        nc.scalar.activation(qk[:, m, n * 512:(n + 1) * 512], ps,
                                             mybir.ActivationFunctionType.Identity,
                                             bias=qkb_sb[:, m:m + 1])
                vT = vtpool.tile([128, PT, C], MM_DT, tag="vT")
                for p in range(PT):
                    ps = mmps.tile([128, 512], F32, tag="mm")
                    for t in range(CT):
                        nc.tensor.matmul(ps,
                                         lhsT=xn[:, t, p * 128:(p + 1) * 128],
                                         rhs=wv_sb[:, t],
                                         start=(t == 0), stop=(t == CT - 1))
                    nc.scalar.activation(vT[:, p], ps, mybir.ActivationFunctionType.Copy)
                return qk, vT

            def att_part2(bb, qk, vT):
                """scores, softmax, attn@v, out projection, residual, store."""
                xt = xts.pop(bb)
                expT = expool.tile([128, PT, HW], MM_DT, tag="expT")
                colps = colpool.tile([128, HW], F32, tag="colps")
                for jm in range(PT):
                    for n in range(NB):
                        ps = mmps.tile([128, 512], F32, tag="mm")
                        for t in range(CT):
                            nc.tensor.matmul(ps,
                                             lhsT=qk[:, CT + t, jm * 128:(jm + 1) * 128],
                                             rhs=qk[:, t, n * 512:(n + 1) * 512],
                                             start=(t == 0), stop=(t == CT - 1))
                        nc.scalar.activation(expT[:, jm, n * 512:(n + 1) * 512], ps,
                                             mybir.ActivationFunctionType.Exp,
                                             scale=SCALE)
                # column sums after the scores loop: by the time these issue,
                # their exp inputs are long done, so the PE never stalls on ACT
                for n in range(NB):
                    for jm in range(PT):
                        nc.tensor.matmul(colps[:, n * 512:(n + 1) * 512],
                                         lhsT=ones_sb,
                                         rhs=expT[:, jm, n * 512:(n + 1) * 512],
                                         start=(jm == 0), stop=(jm == PT - 1))
                recip = rpool.tile([128, HW], F32, tag="recip")
                nc.vector.reciprocal(recip, colps)

                oun = ounpool.tile([128, CT, HW], MM_DT, tag="oun")
                for m in range(CT):
                    for n in range(NB):
                        ps = mmps.tile([128, 512], F32, tag="mm")
                        for jm in range(PT):
                            nc.tensor.matmul(ps,
                                             lhsT=vT[:, jm, m * 128:(m + 1) * 128],
                                             rhs=expT[:, jm, n * 512:(n + 1) * 512],
                                             start=(jm == 0), stop=(jm == PT - 1))
                        nc.vector.tensor_copy(oun[:, m, n * 512:(n + 1) * 512], ps)

                for m in range(CT):
                    for n in range(NB):
                        ps = mmps.tile([128, 512], F32, tag="mm")
                        for t in range(CT):
                            nc.tensor.matmul(ps,
                                             lhsT=wo_sb[:, t, m * 128:(m + 1) * 128],
                                             rhs=oun[:, t, n * 512:(n + 1) * 512],
                                             start=(t == 0), stop=(t == CT - 1))
                        ftmp = ftpool.tile([128, 512], F32, tag="ftmp")
                        nc.scalar.activation(ftmp, ps, mybir.ActivationFunctionType.Identity,
                                             bias=outb_sb[:, m:m + 1])
                        nc.vector.tensor_mul(ftmp, ftmp, recip[:, n * 512:(n + 1) * 512])
                        nc.vector.tensor_add(xt[:, m, n * 512:(n + 1) * 512], ftmp,
                                             xt[:, m, n * 512:(n + 1) * 512])
                        nc.sync.dma_start(
                            out=out_d[bb, m * 128:(m + 1) * 128, n * 512:(n + 1) * 512],
                            in_=xt[:, m, n * 512:(n + 1) * 512])

            # ---- software pipeline over batches ----
            xn_cur = norm_stage(0)
            for bb in range(B_PER_CORE):
                if bb + 1 < B_PER_CORE:
                    load_x(bb + 1)
                qk, vT = att_part1(bb, xn_cur)
                if bb + 1 < B_PER_CORE:
                    xn_next = norm_stage(bb + 1)
                else:
                    xn_next = None
                att_part2(bb, qk, vT)
                xn_cur = xn_next
    return nc


_NC_CACHE = None


def kernel(x, norm_gamma, norm_beta, qkv_w, qkv_b, out_w, out_b):
    global _NC_CACHE
    if _NC_CACHE is None:
        _NC_CACHE = build_nc()
    nc = _NC_CACHE

    x = np.ascontiguousarray(np.asarray(x, np.float32).reshape(B_TOTAL, C, HW))
    qkv_w = np.asarray(qkv_w, np.float32)
    out_w = np.asarray(out_w, np.float32)
    wqkT = np.ascontiguousarray(qkv_w[: 2 * C].T)
    wvT = np.ascontiguousarray(qkv_w[2 * C:].T)
    woT = np.ascontiguousarray(out_w.T)
    qkb = np.ascontiguousarray(np.asarray(qkv_b, np.float32)[: 2 * C])
    # v-bias contributes out_w @ bv to every pixel (softmax rows sum to 1)
    outb = np.ascontiguousarray(
        np.asarray(out_b, np.float32) + out_w @ np.asarray(qkv_b, np.float32)[2 * C:])
    gamma = np.ascontiguousarray(np.asarray(norm_gamma, np.float32))
    beta = np.ascontiguousarray(np.asarray(norm_beta, np.float32))
    cidx = np.arange(C)
    # each group = 64 channels; selector averages the 64 per-channel stats
    sel = np.ascontiguousarray((cidx[:, None] // (C // GROUPS) == np.arange(GROUPS)[None, :])
                               .astype(np.float32) / (C // GROUPS))
    selT = np.ascontiguousarray((np.arange(GROUPS)[:, None] == cidx[None, :] // (C // GROUPS))
                                .astype(np.float32))

    shared = {"wqkT": wqkT, "wvT": wvT, "woT": woT, "qkb": qkb, "outb": outb,
              "gamma": gamma, "beta": beta, "sel": sel, "selT": selT}
    in_maps = [{"x": x[c * B_PER_CORE:(c + 1) * B_PER_CORE], **shared}
               for c in range(N_CORES)]

    trace = bool(int(os.environ.get("KERNEL_TRACE", "0")))
    res = run_bass_kernel_spmd(nc, in_maps, list(range(N_CORES)), trace=trace)
    if trace and res.exec_time_ns is not None:
        print(f"HW exec time: {res.exec_time_ns} ns")
        print(f"(mean across cores: {res.mean_exec_time_ns} ns, "
              f"max core: {res.max_exec_time_core_id})")

    out = np.concatenate([res.results[c]["out"] for c in range(N_CORES)], axis=0)
    return out.reshape(B_TOTAL, C, 32, 32).astype(np.float32)


# revision 20
# speedup vs baseline: 1.0227x; 1.0227x over previous
"""Trainium2 Bass kernel: GroupNorm + single-head self-attention block.

Reference computation (per batch b, x: [C=512, HW=1024] after flattening spatial):
    xn   = groupnorm(x, 8 groups over C, eps=1e-5) * gamma + beta
    qkv  = qkv_w @ xn + qkv_b            # [3C, HW]
    sT   = k^T q * C^-0.5                # computed transposed: [j, i]
    e    = exp(sT)                       # softmax without max-subtraction
    colsum[i] = sum_j e[j, i]
    oun  = v @ e                         # [C, i]
    res  = out_w @ oun + out_b
    out  = x + res / colsum              # residual; softmax denom applied last

Sharding: data-parallel over batch, 32 batches / 8 cores = 4 per core.
All large matmuls run as float32r (fp32 storage, single-pass PE streaming);
measured end-to-end absmax error vs the fp32 reference is ~3e-3 under a
worst-case bf16-rounding model of float32r.
"""

import json
import os

import numpy as np

import concourse.bass as bass
import concourse.mybir as mybir
import concourse.tile as tile
from concourse.bass_utils import run_bass_kernel_spmd


def _spill_multiwaits(raw: bytes) -> bytes:
    """Walrus in this toolchain accepts only one sync-wait command per
    instruction descriptor. Spill extra on_wait entries onto single-wait
    EventSemaphore instructions inserted immediately before, on the same
    engine queue (the exact pattern Tile's own barriers use), which is
    semantically identical: the queue blocks at the same point either way.
    """
    j = json.loads(raw)
    n = 0
    for fn in j.get("functions", []):
        for blk in fn.get("blocks", []):
            out = []
            for inst in blk.get("instructions", []):
                si = inst.get("sync_info") or {}
                waits = si.get("on_wait") or []
                if len(waits) > 1 and inst.get("engine"):
                    for spilled in waits[:-1]:
                        n += 1
                        out.append({
                            "debug": inst.get("debug", 0),
                            "engine": inst["engine"],
                            "ins": [],
                            "name": f"{inst['name']}-sw{n}",
                            "opcode": "EventSemaphore",
                            "outs": [],
                            "sync_info": {"on_update": [], "on_wait": [spilled]},
                        })
                    si["on_wait"] = waits[-1:]
                out.append(inst)
            blk["instructions"] = out
    return json.dumps(j).encode()


_orig_to_json_bytes = bass.Bass.to_json_bytes


def _patched_to_json_bytes(self):
    return _spill_multiwaits(_orig_to_json_bytes(self))


bass.Bass.to_json_bytes = _patched_to_json_bytes

F32 = mybir.dt.float32
MM_DT = mybir.dt.float32r

N_CORES = 8
B_TOTAL = 32
B_PER_CORE = B_TOTAL // N_CORES
C = 512
HW = 1024
GROUPS = 8
EPS = 1e-5
SCALE = float(C) ** -0.5

CT = C // 128   # 4 channel tiles
PT = HW // 128  # 8 pixel tiles
NB = HW // 512  # 2 free-dim blocks of 512


def build_nc():
    nc = bass.Bass()

    x_d = nc.dram_tensor("x", [B_PER_CORE, C, HW], F32, kind="ExternalInput")
    wqkT_d = nc.dram_tensor("wqkT", [C, 2 * C], MM_DT, kind="ExternalInput")
    wvT_d = nc.dram_tensor("wvT", [C, C], MM_DT, kind="ExternalInput")
    woT_d = nc.dram_tensor("woT", [C, C], MM_DT, kind="ExternalInput")
    qkb_d = nc.dram_tensor("qkb", [2 * C], F32, kind="ExternalInput")
    outb_d = nc.dram_tensor("outb", [C], F32, kind="ExternalInput")
    gamma_d = nc.dram_tensor("gamma", [C], F32, kind="ExternalInput")
    beta_d = nc.dram_tensor("beta", [C], F32, kind="ExternalInput")
    sel_d = nc.dram_tensor("sel", [C, GROUPS], F32, kind="ExternalInput")
    selT_d = nc.dram_tensor("selT", [GROUPS, C], F32, kind="ExternalInput")
    out_d = nc.dram_tensor("out", [B_PER_CORE, C, HW], F32, kind="ExternalOutput")
    warmdump_d = nc.dram_tensor("warmdump", [128, 4], F32)

    with tile.TileContext(nc) as tc:
        with (
            tc.tile_pool(name="wpool", bufs=1) as wpool,
            tc.tile_pool(name="xpool", bufs=2) as xpool,
            tc.tile_pool(name="xnpool", bufs=1) as xnpool,
            tc.tile_pool(name="qkpool", bufs=1) as qkpool,
            tc.tile_pool(name="vtpool", bufs=1) as vtpool,
            tc.tile_pool(name="expool", bufs=1) as expool,
            tc.tile_pool(name="ounpool", bufs=1) as ounpool,
            tc.tile_pool(name="rpool", bufs=1) as rpool,
            tc.tile_pool(name="spool", bufs=2) as spool,
            tc.tile_pool(name="ftpool", bufs=2) as ftpool,
            tc.tile_pool(name="mmps", bufs=4, space=bass.MemorySpace.PSUM) as mmps,
            tc.tile_pool(name="colps", bufs=1, space=bass.MemorySpace.PSUM) as colpool,
            tc.tile_pool(name="stps", bufs=1, space=bass.MemorySpace.PSUM) as stps,
        ):
            xts = {}

            def load_x(bb):
                xt = xpool.tile([128, CT, HW], F32, tag="xt")
                xts[bb] = xt
                # per-c-tile chunks so bn_stats can start before the full load
                for t in range(CT):
                    nc.sync.dma_start(
                        out=xt[:, t],
                        in_=x_d[bb, t * 128:(t + 1) * 128, :])
                return xt

            # x(0) first: its consumer chain (stats -> xn -> qkv) is the
            # critical path to the first big matmul
            load_x(0)

            # ---- tiny constants (cheap DMAs / memsets) ----
            eps_sb = wpool.tile([128, 1], F32)
            nc.vector.memset(eps_sb, EPS)
            ones_st = wpool.tile([128, 128], F32)
            nc.vector.memset(ones_st, 1.0)
            ones_sb = wpool.tile([128, 128], MM_DT)
            nc.vector.tensor_copy(ones_sb, ones_st)
            # HAM warmup: keep the PE busy under the startup DMA window so the
            # clock gate reaches 8/8 (2.4GHz) before the first real matmul,
            # instead of ~40 real matmuls running at the cold 1.2GHz rate.
            warm_st = wpool.tile([128, 512], F32)
            nc.vector.memset(warm_st, 0.0)
            warm_rhs = wpool.tile([128, 512], MM_DT)
            nc.vector.tensor_copy(warm_rhs, warm_st)
            warm_ps = stps.tile([128, 512], F32, tag="gps")
            for w in range(35):
                nc.tensor.matmul(warm_ps, lhsT=ones_sb, rhs=warm_rhs,
                                 start=True, stop=True)
            warm_out = wpool.tile([128, 4], F32)
            nc.vector.tensor_copy(warm_out, warm_ps[:, 0:4])
            nc.sync.dma_start(out=warmdump_d[:, :], in_=warm_out)
            sel_st = wpool.tile([128, CT, GROUPS], F32)
            nc.sync.dma_start(out=sel_st, in_=sel_d.rearrange("(t p) g -> p t g", p=128))
            sel_sb = wpool.tile([128, CT, GROUPS], MM_DT)
            nc.vector.tensor_copy(sel_sb, sel_st)
            selT_st = wpool.tile([GROUPS, C], F32)
            nc.sync.dma_start(out=selT_st, in_=selT_d[:, :])
            selT_sb = wpool.tile([GROUPS, C], MM_DT)
            nc.vector.tensor_copy(selT_sb, selT_st)
            qkb_sb = wpool.tile([128, 2 * CT], F32)
            nc.sync.dma_start(out=qkb_sb, in_=qkb_d.rearrange("(m p) -> p m", p=128))
            outb_sb = wpool.tile([128, CT], F32)
            nc.sync.dma_start(out=outb_sb, in_=outb_d.rearrange("(m p) -> p m", p=128))
            gamma_sb = wpool.tile([128, CT], F32)
            nc.sync.dma_start(out=gamma_sb, in_=gamma_d.rearrange("(m p) -> p m", p=128))
            beta_sb = wpool.tile([128, CT], F32)
            nc.sync.dma_start(out=beta_sb, in_=beta_d.rearrange("(m p) -> p m", p=128))

            # ---- weights (split per c-tile for queue parallelism) ----
            wqk_sb = wpool.tile([128, CT, 2 * C], MM_DT)
            wv_sb = wpool.tile([128, CT, C], MM_DT)
            wo_sb = wpool.tile([128, CT, C], MM_DT)
            wqkT_r = wqkT_d.rearrange("(t p) o -> p t o", p=128)
            wvT_r = wvT_d.rearrange("(t p) o -> p t o", p=128)
            woT_r = woT_d.rearrange("(t p) o -> p t o", p=128)
            for t in range(CT):
                nc.sync.dma_start(out=wqk_sb[:, t], in_=wqkT_r[:, t])
            for t in range(CT):
                nc.sync.dma_start(out=wv_sb[:, t], in_=wvT_r[:, t])
                nc.sync.dma_start(out=wo_sb[:, t], in_=woT_r[:, t])

            def norm_stage(bb):
                """GroupNorm stats + normalized activations for batch bb."""
                xt = xts[bb]
                stats3 = spool.tile([128, CT, 4], F32, tag="stats3")
                nc.vector.memset(stats3, 0.0)
                for t in range(CT):
                    st6 = spool.tile([128, 2, 6], F32, tag="st6")
                    for sg in range(2):
                        nc.vector.bn_stats(out=st6[:, sg], in_=xt[:, t, sg * 512:(sg + 1) * 512])
                    nc.vector.bn_aggr(out=stats3[:, t, 0:2], in_=st6)
                    nc.vector.tensor_mul(stats3[:, t, 2:3], stats3[:, t, 0:1], stats3[:, t, 0:1])
                stats3r = spool.tile([128, CT, 4], MM_DT, tag="stats3r")
                nc.vector.tensor_copy(stats3r, stats3)
                gps = stps.tile([GROUPS, 4], F32, tag="gps")
                for t in range(CT):
                    nc.tensor.matmul(gps, lhsT=sel_sb[:, t], rhs=stats3r[:, t],
                                     start=(t == 0), stop=(t == CT - 1))
                # group var = E[var_c] + E[mean_c^2] - E[mean_c]^2 ; then rstd
                gsb = spool.tile([GROUPS, 4], F32, tag="gsb")
                nc.vector.tensor_copy(gsb, gps)
                gs = spool.tile([GROUPS, 4], F32, tag="gs")
                nc.vector.memset(gs, 0.0)
                tmp8 = spool.tile([GROUPS, 1], F32, tag="tmp8")
                nc.vector.tensor_mul(tmp8, gsb[:, 0:1], gsb[:, 0:1])
                nc.vector.tensor_add(gs[:, 1:2], gsb[:, 1:2], gsb[:, 2:3])
                nc.vector.tensor_sub(gs[:, 1:2], gs[:, 1:2], tmp8)
                nc.scalar.activation(gs[:, 1:2], gs[:, 1:2],
                                     mybir.ActivationFunctionType.Sqrt,
                                     bias=eps_sb[:GROUPS])
                nc.vector.reciprocal(gs[:, 1:2], gs[:, 1:2])
                nc.vector.tensor_copy(gs[:, 0:1], gsb[:, 0:1])
                # broadcast group stats back to channel partitions
                gsr = spool.tile([GROUPS, 4], MM_DT, tag="gsr")
                nc.vector.tensor_copy(gsr, gs)
                csps = stps.tile([128, CT, 4], F32, tag="csps")
                for t in range(CT):
                    nc.tensor.matmul(csps[:, t], lhsT=selT_sb[:, t * 128:(t + 1) * 128],
                                     rhs=gsr, start=True, stop=True)
                # per-channel affine: xn = x * s + tt
                stv = spool.tile([128, CT, 2], F32, tag="stv")
                for t in range(CT):
                    tmpc = spool.tile([128, 1], F32, tag="tmpc")
                    nc.vector.tensor_mul(stv[:, t, 0:1], csps[:, t, 1:2], gamma_sb[:, t:t + 1])
                    nc.vector.tensor_mul(tmpc, csps[:, t, 0:1], stv[:, t, 0:1])
                    nc.vector.tensor_sub(stv[:, t, 1:2], beta_sb[:, t:t + 1], tmpc)
                xn = xnpool.tile([128, CT, HW], MM_DT, tag="xn")
                for t in range(CT):
                    nc.vector.tensor_scalar(out=xn[:, t], in0=xt[:, t],
                                            scalar1=stv[:, t, 0:1], scalar2=stv[:, t, 1:2],
                                            op0=mybir.AluOpType.mult,
                                            op1=mybir.AluOpType.add)
                return xn

            def att_part1(bb, xn):
                """qkv projections for batch bb."""
                qk = qkpool.tile([128, 2 * CT, HW], MM_DT, tag="qk")
                for m in range(2 * CT):
                    for n in range(NB):
                        ps = mmps.tile([128, 512], F32, tag="mm")
                        for t in range(CT):
                            nc.tensor.matmul(ps,
                                             lhsT=wqk_sb[:, t, m * 128:(m + 1) * 128],
                                             rhs=xn[:, t, n * 512:(n + 1) * 512],
                                             start=(t == 0), stop=(t == CT - 1))
                        nc.scalar.activation(qk[:, m, n * 512:(n + 1) * 512], ps,
                                             mybir.ActivationFunctionType.Identity,
                                             bias=qkb_sb[:, m:m + 1])
                vT = vtpool.tile([128, PT, C], MM_DT, tag="vT")
                for p in range(PT):
                    ps = mmps.tile([128, 512], F32, tag="mm")
                    for t in range(CT):
                        nc.tensor.matmul(ps,
                                         lhsT=xn[:, t, p * 128:(p + 1) * 128],
                                         rhs=wv_sb[:, t],
                                         start=(t == 0), stop=(t == CT - 1))
                    nc.scalar.activation(vT[:, p], ps, mybir.ActivationFunctionType.Copy)
                return qk, vT

            def att_part2(bb, qk, vT):
                """scores, softmax, attn@v, out projection, residual, store."""
                xt = xts.pop(bb)
                expT = expool.tile([128, PT, HW], MM_DT, tag="expT")
                colps = colpool.tile([128, HW], F32, tag="colps")
                for jm in range(PT):
                    for n in range(NB):
                        ps = mmps.tile([128, 512], F32, tag="mm")
                        for t in range(CT):
                            nc.tensor.matmul(ps,
                                             lhsT=qk[:, CT + t, jm * 128:(jm + 1) * 128],
                                             rhs=qk[:, t, n * 512:(n + 1) * 512],
                                             start=(t == 0), stop=(t == CT - 1))
                        nc.scalar.activation(expT[:, jm, n * 512:(n + 1) * 512], ps,
                                             mybir.ActivationFunctionType.Exp,
                                             scale=SCALE)
                # column sums after the scores loop: by the time these issue,
                # their exp inputs are long done, so the PE never stalls on ACT
                for n in range(NB):
                    for jm in range(PT):
                        nc.tensor.matmul(colps[:, n * 512:(n + 1) * 512],
                                         lhsT=ones_sb,
                                         rhs=expT[:, jm, n * 512:(n + 1) * 512],
                                         start=(jm == 0), stop=(jm == PT - 1))
                recip = rpool.tile([128, HW], F32, tag="recip")
                nc.vector.reciprocal(recip, colps)

                oun = ounpool.tile([128, CT, HW], MM_DT, tag="oun")
                for m in range(CT):
                    for n in range(NB):
                        ps = mmps.tile([128, 512], F32, tag="mm")
                        for jm in range(PT):
                            nc.tensor.matmul(ps,
                                             lhsT=vT[:, jm, m * 128:(m + 1) * 128],
                                             rhs=expT[:, jm, n * 512:(n + 1) * 512],
                                             start=(jm == 0), stop=(jm == PT - 1))
                        nc.vector.tensor_copy(oun[:, m, n * 512:(n + 1) * 512], ps)

                for m in range(CT):
                    for n in range(NB):
                        ps = mmps.tile([128, 512], F32, tag="mm")
                        for t in range(CT):
                            nc.tensor.matmul(ps,
                                             lhsT=wo_sb[:, t, m * 128:(m + 1) * 128],
                                             rhs=oun[:, t, n * 512:(n + 1) * 512],
                                             start=(t == 0), stop=(t == CT - 1))
                        ftmp = ftpool.tile([128, 512], F32, tag="ftmp")
                        nc.scalar.activation(ftmp, ps, mybir.ActivationFunctionType.Identity,
                                             bias=outb_sb[:, m:m + 1])
                        nc.vector.tensor_mul(ftmp, ftmp, recip[:, n * 512:(n + 1) * 512])
                        nc.vector.tensor_add(xt[:, m, n * 512:(n + 1) * 512], ftmp,
                                             xt[:, m, n * 512:(n + 1) * 512])
                        nc.sync.dma_start(
                            out=out_d[bb, m * 128:(m + 1) * 128, n * 512:(n + 1) * 512],
                            in_=xt[:, m, n * 512:(n + 1) * 512])

            # ---- software pipeline over batches ----
            xn_cur = norm_stage(0)
            for bb in range(B_PER_CORE):
                if bb + 1 < B_PER_CORE:
                    load_x(bb + 1)
                qk, vT = att_part1(bb, xn_cur)
                if bb + 1 < B_PER_CORE:
                    xn_next = norm_stage(bb + 1)
                else:
                    xn_next = None
                att_part2(bb, qk, vT)
                xn_cur = xn_next
    return nc


_NC_CACHE = None


def kernel(x, norm_gamma, norm_beta, qkv_w, qkv_b, out_w, out_b):
    global _NC_CACHE
    if _NC_CACHE is None:
        _NC_CACHE = build_nc()
    nc = _NC_CACHE

    x = np.ascontiguousarray(np.asarray(x, np.float32).reshape(B_TOTAL, C, HW))
    qkv_w = np.asarray(qkv_w, np.float32)
    out_w = np.asarray(out_w, np.float32)
    wqkT = np.ascontiguousarray(qkv_w[: 2 * C].T)
    wvT = np.ascontiguousarray(qkv_w[2 * C:].T)
    woT = np.ascontiguousarray(out_w.T)
    qkb = np.ascontiguousarray(np.asarray(qkv_b, np.float32)[: 2 * C])
    # v-bias contributes out_w @ bv to every pixel (softmax rows sum to 1)
    outb = np.ascontiguousarray(
        np.asarray(out_b, np.float32) + out_w @ np.asarray(qkv_b, np.float32)[2 * C:])
    gamma = np.ascontiguousarray(np.asarray(norm_gamma, np.float32))
    beta = np.ascontiguousarray(np.asarray(norm_beta, np.float32))
    cidx = np.arange(C)
    # each group = 64 channels; selector averages the 64 per-channel stats
    sel = np.ascontiguousarray((cidx[:, None] // (C // GROUPS) == np.arange(GROUPS)[None, :])
                               .astype(np.float32) / (C // GROUPS))
    selT = np.ascontiguousarray((np.arange(GROUPS)[:, None] == cidx[None, :] // (C // GROUPS))
                                .astype(np.float32))

    shared = {"wqkT": wqkT, "wvT": wvT, "woT": woT, "qkb": qkb, "outb": outb,
              "gamma": gamma, "beta": beta, "sel": sel, "selT": selT}
    in_maps = [{"x": x[c * B_PER_CORE:(c + 1) * B_PER_CORE], **shared}
               for c in range(N_CORES)]

    trace = bool(int(os.environ.get("KERNEL_TRACE", "0")))
    res = run_bass_kernel_spmd(nc, in_maps, list(range(N_CORES)), trace=trace)
    if trace and res.exec_time_ns is not None:
        print(f"HW exec time: {res.exec_time_ns} ns")
        print(f"(mean across cores: {res.mean_exec_time_ns} ns, "
              f"max core: {res.max_exec_time_core_id})")

    out = np.concatenate([res.results[c]["out"] for c in range(N_CORES)], axis=0)
    return out.reshape(B_TOTAL, C, 32, 32).astype(np.float32)
